# revision 28
# baseline (speedup 1.0000x reference)
"""Trainium2 Bass kernel for DGLBatchCapsuleLayer (capsule dynamic routing).

Math (reference):
    u_hat[c,j,b,t] = sum_i W[c,j,t,i] * x[b,i,c]
    3 routing iterations:
        c_ij = softmax_j(b_ij)
        s[j,b,t] = sum_c c_ij[c,j] * u_hat[c,j,b,t]
        v = squash_t(s)
        b_ij += mean_b <v_j, u_hat_cj> (skipped on last iter - unused)
    out = v as [B, J, S, 1]

u_hat (360 MB) is never materialized; both routing contractions are
matmuls against x with k=(c,i):
    s[b,(j,t)]  = sum_k x[b,k] * (c*W)[k,(j,t)]
    M[k,(j,t)]  = sum_b x[b,k] * v[b,(j,t)]
    bdelta[c,j] = sum_{t,i} W[k,(j,t)] * M[k,(j,t)]
b_ij lives in an i-replicated (c,i) layout; the i-reduction +
replication of bdelta is one matmul against a block-diag 8x8-ones
matrix.

Distribution: iterations 1-2 (which feed the cross-core-coupled b_ij
update) run fully REPLICATED on all 8 cores - a 46KB AllReduce costs
~40us + ~100us multi-core launch-skew exposure here (measured), far
more than replicating the compute. Iteration 3 has no b_ij update, so
each core computes only its own B/8 output shard; the host concatenates.

Schedule: the PE instruction stream is software-pipelined - the next
iteration's s-matmul k-octets are emitted BETWEEN the b-update q-groups
of the current iteration, so the in-order PE queue never idles on the
softmax->c*W chain or on x2 streaming. The b-update runs in 3 k-slices,
each followed by its own bd-matmul + softmax + (c broadcast)*(W) chain
(half-q-granular scalar broadcast + DVE 2x multiply, so c*W rows
unblock the next s-phase in small waves). Concurrent PSUM accumulation
groups must live in DISTINCT banks (interleaving two open accumulation
groups in one bank corrupts both - measured), so the 4 s-accumulators
and the M-pack accumulators each get a private bank; mpsum runs 3 deep
so the PE can run ahead of the DVE PSUM drain. The W-mult/t-reduce
drain is one fat tensor_mul per 3-chunk pack plus one merged per-q
reduce. Input DMAs are chunked and explicitly ordered (wtf_q/xtf_q
pairs first, x2 batches in consumption order, xtb between them) so the
first s-matmul starts ~10us in; the kernel sits near the ~300GB/s
input-streaming floor through iteration 0 and is PE/chain-latency
bound after.
"""

import numpy as np
import ml_dtypes

NCORES = 8
B, I, C, J, S = 512, 8, 1152, 10, 16
BSH = B // NCORES          # 64-row output shard per core
KF = C * I                 # 9216 full contraction length, k = c*8+i
NKF = KF // 128            # 72 k-chunks
NQ = 9                     # q-groups (8 k-chunks each)
NB = B // 128              # 4 batch chunks
JT = J * S                 # 160
QSL = 3                    # q's per b-update slice
KSL = QSL * 8              # 24 k-chunks per slice
NSL = NKF // KSL           # 3 slices
GORD = (0, 2, 1, 3)        # bank-alternating psum write order

_BF16 = ml_dtypes.bfloat16

_built = None


def _build():
    import concourse.bass as bass  # noqa: F401
    import concourse.bacc as bacc
    import concourse.mybir as mybir
    import concourse.tile as tile

    f32 = mybir.dt.float32
    f16 = mybir.dt.float16
    bf16 = mybir.dt.bfloat16
    mult = mybir.AluOpType.mult
    add = mybir.AluOpType.add
    subtract = mybir.AluOpType.subtract
    AX = mybir.AxisListType.X
    Exp = mybir.ActivationFunctionType.Exp

    nc = bacc.Bacc(
        "TRN2",
        target_bir_lowering=False,
        debug=False,
        num_devices=NCORES,
    )

    xtf_d = nc.dram_tensor("xtf", [128, NQ, 8, B], bf16,
                           kind="ExternalInput")
    wtf_d = nc.dram_tensor("wtf", [128, NQ, 8, J, S], bf16,
                           kind="ExternalInput")
    x2f_d = nc.dram_tensor("x2f", [128, NQ, NB, 8 * 128], bf16,
                           kind="ExternalInput")
    xtb_d = nc.dram_tensor("xtb", [128, NKF, BSH], bf16,
                           kind="ExternalInput")
    out_d = nc.dram_tensor("out", [BSH, JT], f32, kind="ExternalOutput")

    rep_np = np.kron(np.eye(16), np.ones((8, 8))).astype(_BF16)
    rep_d = nc.inline_tensor(rep_np, name="repind")

    with tile.TileContext(nc) as tc:
        with (
            tc.tile_pool(name="pers", bufs=1) as pers,
            tc.tile_pool(name="work", bufs=2) as work,
            tc.tile_pool(name="wcp", bufs=2) as wcp,
            tc.tile_pool(name="cex", bufs=2) as cex,
            tc.tile_pool(name="x2rot", bufs=5) as x2rot,
            tc.tile_pool(name="tmprot", bufs=3) as tmprot,
            tc.tile_pool(name="spsum", bufs=1, space="PSUM") as spsum,
            tc.tile_pool(name="mpsum", bufs=3, space="PSUM") as mpsum,
            tc.tile_pool(name="bpsum", bufs=1, space="PSUM") as bpsum,
        ):
            # ---- persistent SBUF ----
            xtf_t = [pers.tile([128, 8, B], bf16, name=f"xtf{q}")
                     for q in range(NQ)]
            wtf_t = [pers.tile([128, 8, J, S], bf16, name=f"wtf{q}")
                     for q in range(NQ)]
            xtb_sb = pers.tile([128, NKF, BSH], bf16, name="xtb_sb")
            rep_sb = pers.tile([128, 128], bf16, name="rep_sb")
            b_rep = pers.tile([128, NKF, J], f32, name="b_rep")
            tjas = [pers.tile([128, KSL, J], bf16, name=f"tja{s3}")
                    for s3 in range(NSL)]
            s16 = pers.tile([128, NB, J, S], f16, name="s16")
            v_bf = pers.tile([128, NB, JT], bf16, name="v_bf")
            vout = pers.tile([BSH, JT], f32, name="vout")

            # ---- input DMAs: wtf/xtf pairs first, rep last ----
            nc.sync.dma_start(wtf_t[0][:, 0:4], wtf_d.ap()[:, 0, 0:4])
            nc.sync.dma_start(xtf_t[0][:, 0:4], xtf_d.ap()[:, 0, 0:4])
            nc.sync.dma_start(wtf_t[0][:, 4:8], wtf_d.ap()[:, 0, 4:8])
            nc.sync.dma_start(xtf_t[0][:, 4:8], xtf_d.ap()[:, 0, 4:8])
            for q in range(1, NQ):
                nc.sync.dma_start(wtf_t[q][:], wtf_d.ap()[:, q])
                nc.sync.dma_start(xtf_t[q][:], xtf_d.ap()[:, q])
            nc.sync.dma_start(rep_sb[:], rep_d.ap())
            nc.vector.memset(b_rep[:], 0.0)

            x2q = {}

            def issue_x2(r, q0, q1):
                for q in range(q0, q1):
                    t_ = x2rot.tile([128, NB, 8 * 128], bf16,
                                    name="x2p", tag="x2p")
                    nc.sync.dma_start(t_[:], x2f_d.ap()[:, q])
                    x2q[(r, q)] = t_

            issue_x2(0, 0, NQ)

            # s-phase psum: one bank per batch chunk (concurrent psum
            # accumulation groups must live in distinct banks)
            sps = [spsum.tile([128, JT], f32, name=f"sp{g}",
                              tag=f"sp{g}") for g in range(NB)]

            def sp_ap(g):
                return sps[g][:]

            wc_slices = [None] * NSL

            def rhs_ap(r, k):
                if r == 0:
                    return wtf_t[k // 8][:, k % 8]
                return wc_slices[k // KSL][:, k % KSL]

            # ---- full-batch s-matmul octet (8 k-chunks) ----
            def s_oct(r, o):
                for k in range(o * 8, o * 8 + 8):
                    for g in range(NB):
                        nc.tensor.matmul(
                            sp_ap(g),
                            xtf_t[k // 8][:, k % 8, g * 128:(g + 1) * 128],
                            rhs_ap(r, k),
                            start=(k == 0),
                            stop=(k == NKF - 1),
                        )

            # ---- last-iter s-matmul octet (64-row shard) ----
            def s2_oct(o):
                for k in range(o * 8, o * 8 + 8):
                    nc.tensor.matmul(
                        sps[0][:BSH, :],
                        xtb_sb[:, k, :],
                        rhs_ap(2, k),
                        start=(k == 0),
                        stop=(k == NKF - 1),
                    )

            # ---- squash straight from PSUM (scalar square + DVE) ----
            def cast_squash():
                sq = work.tile([128, NB, J, S], f32, name="sq", tag="sq")
                for g in range(NB):
                    nc.scalar.square(
                        sq[:, g],
                        sp_ap(g).rearrange("p (j t) -> p j t", t=S),
                    )
                msq = work.tile([128, NB, J], f32, name="msq", tag="msq")
                nc.vector.reduce_sum(msq[:], sq[:], axis=AX)
                d1 = work.tile([128, NB, J], f32, name="d1", tag="d1")
                nc.vector.tensor_scalar_add(d1[:], msq[:], 1.0)
                rd1 = work.tile([128, NB, J], f32, name="rd1", tag="rd1")
                nc.vector.reciprocal(rd1[:], d1[:])
                mag = work.tile([128, NB, J], f32, name="mag", tag="mag")
                nc.scalar.sqrt(mag[:], msq[:])
                f2 = work.tile([128, NB, J], f32, name="f2", tag="f2")
                nc.vector.tensor_mul(f2[:], mag[:], rd1[:])
                for g in range(NB):
                    nc.vector.tensor_mul(
                        v_bf[:, g].rearrange("p (j t) -> p j t", t=S),
                        sp_ap(g).rearrange("p (j t) -> p j t", t=S),
                        f2[:, g].unsqueeze(2).broadcast_to([128, J, S]),
                    )

            # ---- one b-update q-group: M matmuls + W-mult + t-reduce ----
            def m_q(r, s3, qh):
                q = s3 * QSL + qh
                x2p = x2q[(r, q)]
                tmp8 = tmprot.tile([128, 8, J, S], bf16,
                                   name="tmp8", tag="tmp8")
                for c3 in range(3):
                    nkk = min(3, 8 - c3 * 3)
                    mp3 = mpsum.tile([128, 3, JT], f32,
                                     name="mp3", tag="mp3")
                    for kk in range(nkk):
                        kl = c3 * 3 + kk
                        for g in range(NB):
                            nc.tensor.matmul(
                                mp3[:, kk, :],
                                x2p[:, g, kl * 128:(kl + 1) * 128],
                                v_bf[:, g],
                                start=(g == 0),
                                stop=(g == NB - 1),
                            )
                    nc.vector.tensor_mul(
                        tmp8[:, c3 * 3:c3 * 3 + nkk],
                        mp3[:, :nkk].rearrange(
                            "p c (j t) -> p c j t", t=S),
                        wtf_t[q][:, c3 * 3:c3 * 3 + nkk],
                    )
                # one fat reduce per q-group
                # bf16 out is fine: tja only feeds b_ij whose error
                # is already bf16-matmul dominated
                with nc.allow_low_precision(reason="tja bf16"):
                    nc.vector.reduce_sum(
                        tjas[s3][:, qh * 8:qh * 8 + 8],
                        tmp8[:], axis=AX,
                    )

            # ---- chain piece: bd matmul + b update + softmax + wc for
            # k-chunk rows [k0, k1) of slice s3 (whole slice or one q) ----
            def chain_rows(s3, k0, k1, wct):
                n = k1 - k0
                bd = bpsum.tile([128, KSL * J], f32, name="bd", tag="bd")
                nc.tensor.matmul(
                    bd[:, :n * J], rep_sb[:],
                    tjas[s3][:, k0 - s3 * KSL:k1 - s3 * KSL].rearrange(
                        "p k j -> p (k j)"),
                    start=True, stop=True,
                )
                brs = b_rep[:, k0:k1]
                # b_rep += bd * 10/B  (x10 undoes the host-side W/10)
                nc.vector.scalar_tensor_tensor(
                    brs.rearrange("p k j -> p (k j)"),
                    bd[:, :n * J], 10.0 / B,
                    brs.rearrange("p k j -> p (k j)"),
                    op0=mult, op1=add,
                )
                ex = work.tile([128, KSL, J], f32, name="ex", tag="ex")
                nc.scalar.activation(ex[:, :n], brs, Exp)
                den = work.tile([128, KSL], f32, name="den", tag="den")
                nc.vector.reduce_sum(den[:, :n], ex[:, :n], axis=AX)
                rden = work.tile([128, KSL], f32, name="rden", tag="rden")
                nc.vector.reciprocal(rden[:, :n], den[:, :n])
                cb = work.tile([128, KSL, J], bf16, name="cb", tag="cb")
                # cb = (ex * 10) * (1/den)
                nc.vector.scalar_tensor_tensor(
                    cb[:, :n], ex[:, :n], 10.0,
                    rden[:, :n].unsqueeze(2).broadcast_to([128, n, J]),
                    op0=mult, op1=mult,
                )
                for q in range(k0 // 8, k1 // 8):
                    qh = q - s3 * QSL
                    r0 = qh * 8 - (k0 - s3 * KSL)
                    for h in range(2):
                        ce = cex.tile([128, 4, J, S], bf16,
                                      name="ce", tag="ce")
                        nc.scalar.copy(
                            ce[:],
                            cb[:, r0 + h * 4:r0 + h * 4 + 4].unsqueeze(3)
                              .broadcast_to([128, 4, J, S]),
                        )
                        nc.vector.tensor_mul(
                            wct[:, qh * 8 + h * 4:qh * 8 + h * 4 + 4],
                            wtf_t[q][:, h * 4:h * 4 + 4], ce[:],
                        )

            def chain(r, s3):
                wct = wcp.tile([128, KSL, J, S], bf16, name="wc", tag="wc")
                wc_slices[s3] = wct
                chain_rows(s3, s3 * KSL, (s3 + 1) * KSL, wct)

            def chain_q(r, s3, qh, wct):
                q = s3 * QSL + qh
                chain_rows(s3, q * 8, q * 8 + 8, wct)

            def tail():
                sqb = work.tile([BSH, J, S], f32, name="sqb", tag="sqb")
                nc.scalar.square(
                    sqb[:],
                    sps[0][:BSH, :].rearrange("p (j t) -> p j t", t=S))
                msqb = work.tile([BSH, J], f32, name="msqb", tag="msqb")
                nc.vector.reduce_sum(msqb[:], sqb[:], axis=AX)
                d1b = work.tile([BSH, J], f32, name="d1b", tag="d1b")
                nc.vector.tensor_scalar_add(d1b[:], msqb[:], 1.0)
                rd1b = work.tile([BSH, J], f32, name="rd1b", tag="rd1b")
                nc.vector.reciprocal(rd1b[:], d1b[:])
                magb = work.tile([BSH, J], f32, name="magb", tag="magb")
                nc.scalar.sqrt(magb[:], msqb[:])
                f2b = work.tile([BSH, J], f32, name="f2b", tag="f2b")
                nc.vector.tensor_mul(f2b[:], magb[:], rd1b[:])
                nc.vector.tensor_mul(
                    vout[:].rearrange("p (j t) -> p j t", t=S),
                    sps[0][:BSH, :].rearrange("p (j t) -> p j t", t=S),
                    f2b[:].unsqueeze(2).broadcast_to([BSH, J, S]),
                )
                nc.sync.dma_start(out_d.ap(), vout[:])

            # ---- emission schedule (PE-stream software pipelining) ----
            for o in range(NQ):
                s_oct(0, o)
            cast_squash()                      # v_0

            m_q(0, 0, 0); m_q(0, 0, 1); m_q(0, 0, 2); chain(0, 0)
            m_q(0, 1, 0); m_q(0, 1, 1)
            m_q(0, 1, 2); chain(0, 1)
            issue_x2(1, 0, 3)
            s_oct(1, 0)
            m_q(0, 2, 0); s_oct(1, 1)
            m_q(0, 2, 1); s_oct(1, 2)
            m_q(0, 2, 2); chain(0, 2)
            issue_x2(1, 3, NQ)
            nc.sync.dma_start(xtb_sb[:], xtb_d.ap())
            s_oct(1, 3); s_oct(1, 4)
            s_oct(1, 5)
            s_oct(1, 6); s_oct(1, 7); s_oct(1, 8)
            cast_squash()                      # v_1

            m_q(1, 0, 0); m_q(1, 0, 1); m_q(1, 0, 2); chain(1, 0)
            m_q(1, 1, 0); m_q(1, 1, 1)
            m_q(1, 1, 2); chain(1, 1)
            s2_oct(0)
            m_q(1, 2, 0); s2_oct(1)
            m_q(1, 2, 1); s2_oct(2); s2_oct(3)
            m_q(1, 2, 2); chain(1, 2)
            s2_oct(4); s2_oct(5); s2_oct(6); s2_oct(7); s2_oct(8)
            tail()

    nc.compile()
    return nc


def _get_built():
    global _built
    if _built is None:
        _built = _build()
    return _built


def _prep_inputs(x, weight):
    x = np.asarray(x, dtype=np.float32)
    weight = np.asarray(weight, dtype=np.float32)
    xtf = np.ascontiguousarray(x.transpose(2, 1, 0)).reshape(KF, B)
    xtf_m = np.ascontiguousarray(
        xtf.reshape(NKF, 128, B).transpose(1, 0, 2)
    ).astype(_BF16)                       # [128, NKF, B]
    x2f = np.ascontiguousarray(x.transpose(0, 2, 1)).reshape(B, KF)
    x2f_m = np.ascontiguousarray(
        x2f.reshape(NB, 128, NQ, 8 * 128).transpose(1, 2, 0, 3)
    ).astype(_BF16)                       # [128, NQ, NB, 1024]
    wtf = np.ascontiguousarray(weight.transpose(0, 3, 1, 2)).reshape(KF, JT)
    wtf_m = np.ascontiguousarray(
        (wtf * 0.1).reshape(NKF, 128, J, S).transpose(1, 0, 2, 3)
    ).astype(_BF16)                       # [128, NKF, J, S]
    wtf_q = np.ascontiguousarray(
        wtf_m.reshape(128, NQ, 8, J, S))  # [128, NQ, 8, J, S]
    xtf_q = np.ascontiguousarray(
        xtf_m.reshape(128, NQ, 8, B))     # [128, NQ, 8, B]
    in_maps = []
    for core in range(NCORES):
        in_maps.append({
            "xtf": xtf_q,
            "wtf": wtf_q,
            "x2f": x2f_m,
            "xtb": np.ascontiguousarray(
                xtf_m[:, :, core * BSH:(core + 1) * BSH]),
        })
    return in_maps


def run(x, weight, trace=False, warmup=1, **kw):
    from concourse import bass_utils
    nc = _get_built()
    in_maps = _prep_inputs(x, weight)
    for _ in range(warmup):
        bass_utils.run_bass_kernel_spmd(
            nc, in_maps, core_ids=list(range(NCORES)), trace=False
        )
    res = bass_utils.run_bass_kernel_spmd(
        nc, in_maps, core_ids=list(range(NCORES)), trace=trace, **kw
    )
    out = np.concatenate(
        [np.asarray(res.results[c]["out"], dtype=np.float32)
         for c in range(NCORES)], axis=0,
    )
    return out.reshape(B, J, S, 1), res


def kernel(x, weight):
    out, _ = run(x, weight)
    return out


# revision 30
# speedup vs baseline: 1.0173x; 1.0173x over previous
"""Trainium2 Bass kernel for DGLBatchCapsuleLayer (capsule dynamic routing).

Math (reference):
    u_hat[c,j,b,t] = sum_i W[c,j,t,i] * x[b,i,c]
    3 routing iterations:
        c_ij = softmax_j(b_ij)
        s[j,b,t] = sum_c c_ij[c,j] * u_hat[c,j,b,t]
        v = squash_t(s)
        b_ij += mean_b <v_j, u_hat_cj> (skipped on last iter - unused)
    out = v as [B, J, S, 1]

u_hat (360 MB) is never materialized; both routing contractions are
matmuls against x with k=(c,i):
    s[b,(j,t)]  = sum_k x[b,k] * (c*W)[k,(j,t)]
    M[k,(j,t)]  = sum_b x[b,k] * v[b,(j,t)]
    bdelta[c,j] = sum_{t,i} W[k,(j,t)] * M[k,(j,t)]
b_ij lives in an i-replicated (c,i) layout; the i-reduction +
replication of bdelta is one matmul against a block-diag 8x8-ones
matrix.

Distribution: iterations 1-2 (which feed the cross-core-coupled b_ij
update) run fully REPLICATED on all 8 cores - a 46KB AllReduce costs
~40us + ~100us multi-core launch-skew exposure here (measured), far
more than replicating the compute. Iteration 3 has no b_ij update, so
each core computes only its own B/8 output shard; the host concatenates.

Schedule: the PE instruction stream is software-pipelined - the next
iteration's s-matmul k-octets are emitted BETWEEN the b-update q-groups
of the current iteration, so the in-order PE queue never idles on the
softmax->c*W chain or on x2 streaming. The b-update runs in 3 k-slices,
each followed by its own bd-matmul + softmax + (c broadcast)*(W) chain
(half-q-granular scalar broadcast + DVE 2x multiply, so c*W rows
unblock the next s-phase in small waves). Concurrent PSUM accumulation
groups must live in DISTINCT banks (interleaving two open accumulation
groups in one bank corrupts both - measured), so the 4 s-accumulators
and the M-pack accumulators each get a private bank; mpsum runs 3 deep
so the PE can run ahead of the DVE PSUM drain. The W-mult/t-reduce
drain is one fat tensor_mul per 3-chunk pack plus one merged per-q
reduce. Input DMAs are chunked and explicitly ordered (wtf_q/xtf_q
pairs first, x2 batches in consumption order, xtb between them) so the
first s-matmul starts ~10us in; the kernel sits near the ~300GB/s
input-streaming floor through iteration 0 and is PE/chain-latency
bound after.
"""

import numpy as np
import ml_dtypes

NCORES = 8
B, I, C, J, S = 512, 8, 1152, 10, 16
BSH = B // NCORES          # 64-row output shard per core
KF = C * I                 # 9216 full contraction length, k = c*8+i
NKF = KF // 128            # 72 k-chunks
NQ = 9                     # q-groups (8 k-chunks each)
NB = B // 128              # 4 batch chunks
JT = J * S                 # 160
QSL = 3                    # q's per b-update slice
KSL = QSL * 8              # 24 k-chunks per slice
NSL = NKF // KSL           # 3 slices
GORD = (0, 2, 1, 3)        # bank-alternating psum write order

_BF16 = ml_dtypes.bfloat16

_built = None


def _build():
    import concourse.bass as bass  # noqa: F401
    import concourse.bacc as bacc
    import concourse.mybir as mybir
    import concourse.tile as tile

    f32 = mybir.dt.float32
    f16 = mybir.dt.float16
    bf16 = mybir.dt.bfloat16
    mult = mybir.AluOpType.mult
    add = mybir.AluOpType.add
    subtract = mybir.AluOpType.subtract
    AX = mybir.AxisListType.X
    Exp = mybir.ActivationFunctionType.Exp

    nc = bacc.Bacc(
        "TRN2",
        target_bir_lowering=False,
        debug=False,
        num_devices=NCORES,
    )

    xtf_d = nc.dram_tensor("xtf", [128, NQ, 8, B], bf16,
                           kind="ExternalInput")
    wtf_d = nc.dram_tensor("wtf", [128, NQ, 8, J, S], bf16,
                           kind="ExternalInput")
    x2f_d = nc.dram_tensor("x2f", [128, NQ, NB, 8 * 128], bf16,
                           kind="ExternalInput")
    xtb_d = nc.dram_tensor("xtb", [128, NKF, BSH], bf16,
                           kind="ExternalInput")
    out_d = nc.dram_tensor("out", [BSH, JT], f32, kind="ExternalOutput")

    rep_np = np.kron(np.eye(16), np.ones((8, 8))).astype(_BF16)
    rep_d = nc.inline_tensor(rep_np, name="repind")

    with tile.TileContext(nc) as tc:
        with (
            tc.tile_pool(name="pers", bufs=1) as pers,
            tc.tile_pool(name="work", bufs=2) as work,
            tc.tile_pool(name="wcp", bufs=2) as wcp,
            tc.tile_pool(name="cex", bufs=2) as cex,
            tc.tile_pool(name="x2rot", bufs=5) as x2rot,
            tc.tile_pool(name="tmprot", bufs=3) as tmprot,
            tc.tile_pool(name="spsum", bufs=1, space="PSUM") as spsum,
            tc.tile_pool(name="mpsum", bufs=3, space="PSUM") as mpsum,
            tc.tile_pool(name="bpsum", bufs=1, space="PSUM") as bpsum,
        ):
            # ---- persistent SBUF ----
            xtf_t = [pers.tile([128, 8, B], bf16, name=f"xtf{q}")
                     for q in range(NQ)]
            wtf_t = [pers.tile([128, 8, J, S], bf16, name=f"wtf{q}")
                     for q in range(NQ)]
            xtb_sb = pers.tile([128, NKF, BSH], bf16, name="xtb_sb")
            rep_sb = pers.tile([128, 128], bf16, name="rep_sb")
            b_rep = pers.tile([128, NKF, J], f32, name="b_rep")
            tjas = [pers.tile([128, KSL, J], bf16, name=f"tja{s3}")
                    for s3 in range(NSL)]
            s16 = pers.tile([128, NB, J, S], f16, name="s16")
            v_bf = pers.tile([128, NB, JT], bf16, name="v_bf")
            vout = pers.tile([BSH, JT], f32, name="vout")

            # ---- input DMAs: wtf/xtf pairs first, rep last ----
            nc.sync.dma_start(wtf_t[0][:, 0:4], wtf_d.ap()[:, 0, 0:4])
            nc.sync.dma_start(xtf_t[0][:, 0:4], xtf_d.ap()[:, 0, 0:4])
            nc.sync.dma_start(wtf_t[0][:, 4:8], wtf_d.ap()[:, 0, 4:8])
            nc.sync.dma_start(xtf_t[0][:, 4:8], xtf_d.ap()[:, 0, 4:8])
            for q in range(1, NQ):
                nc.sync.dma_start(wtf_t[q][:], wtf_d.ap()[:, q])
                nc.sync.dma_start(xtf_t[q][:], xtf_d.ap()[:, q])
            nc.sync.dma_start(rep_sb[:], rep_d.ap())
            nc.vector.memset(b_rep[:], 0.0)

            x2q = {}

            def issue_x2(r, q0, q1):
                for q in range(q0, q1):
                    t_ = x2rot.tile([128, NB, 8 * 128], bf16,
                                    name="x2p", tag="x2p")
                    nc.sync.dma_start(t_[:], x2f_d.ap()[:, q])
                    x2q[(r, q)] = t_

            issue_x2(0, 0, NQ)

            # s-phase psum: one bank per batch chunk (concurrent psum
            # accumulation groups must live in distinct banks)
            sps = [spsum.tile([128, JT], f32, name=f"sp{g}",
                              tag=f"sp{g}") for g in range(NB)]

            def sp_ap(g):
                return sps[g][:]

            wc_slices = [None] * NSL

            def rhs_ap(r, k):
                if r == 0:
                    return wtf_t[k // 8][:, k % 8]
                return wc_slices[k // KSL][:, k % KSL]

            # ---- full-batch s-matmul octet (8 k-chunks) ----
            def s_oct(r, o):
                for k in range(o * 8, o * 8 + 8):
                    for g in range(NB):
                        nc.tensor.matmul(
                            sp_ap(g),
                            xtf_t[k // 8][:, k % 8, g * 128:(g + 1) * 128],
                            rhs_ap(r, k),
                            start=(k == 0),
                            stop=(k == NKF - 1),
                        )

            # ---- last-iter s-matmul octet (64-row shard) ----
            def s2_oct(o):
                for k in range(o * 8, o * 8 + 8):
                    nc.tensor.matmul(
                        sps[0][:BSH, :],
                        xtb_sb[:, k, :],
                        rhs_ap(2, k),
                        start=(k == 0),
                        stop=(k == NKF - 1),
                    )

            # ---- squash straight from PSUM (scalar square + DVE) ----
            def cast_squash():
                sq = work.tile([128, NB, J, S], f32, name="sq", tag="sq")
                for g in range(NB):
                    nc.scalar.square(
                        sq[:, g],
                        sp_ap(g).rearrange("p (j t) -> p j t", t=S),
                    )
                msq = work.tile([128, NB, J], f32, name="msq", tag="msq")
                nc.vector.reduce_sum(msq[:], sq[:], axis=AX)
                d1 = work.tile([128, NB, J], f32, name="d1", tag="d1")
                nc.vector.tensor_scalar_add(d1[:], msq[:], 1.0)
                rd1 = work.tile([128, NB, J], f32, name="rd1", tag="rd1")
                nc.vector.reciprocal(rd1[:], d1[:])
                mag = work.tile([128, NB, J], f32, name="mag", tag="mag")
                nc.scalar.sqrt(mag[:], msq[:])
                f2 = work.tile([128, NB, J], f32, name="f2", tag="f2")
                nc.vector.tensor_mul(f2[:], mag[:], rd1[:])
                for g in range(NB):
                    nc.vector.tensor_mul(
                        v_bf[:, g].rearrange("p (j t) -> p j t", t=S),
                        sp_ap(g).rearrange("p (j t) -> p j t", t=S),
                        f2[:, g].unsqueeze(2).broadcast_to([128, J, S]),
                    )

            # ---- one b-update q-group: M matmuls + W-mult + t-reduce ----
            def m_q(r, s3, qh):
                q = s3 * QSL + qh
                x2p = x2q[(r, q)]
                tmp8 = tmprot.tile([128, 8, J, S], bf16,
                                   name="tmp8", tag="tmp8")
                for c3 in range(3):
                    nkk = min(3, 8 - c3 * 3)
                    mp3 = mpsum.tile([128, 3, JT], f32,
                                     name="mp3", tag="mp3")
                    for kk in range(nkk):
                        kl = c3 * 3 + kk
                        for g in range(NB):
                            nc.tensor.matmul(
                                mp3[:, kk, :],
                                x2p[:, g, kl * 128:(kl + 1) * 128],
                                v_bf[:, g],
                                start=(g == 0),
                                stop=(g == NB - 1),
                            )
                    nc.vector.tensor_mul(
                        tmp8[:, c3 * 3:c3 * 3 + nkk],
                        mp3[:, :nkk].rearrange(
                            "p c (j t) -> p c j t", t=S),
                        wtf_t[q][:, c3 * 3:c3 * 3 + nkk],
                    )
                # one fat reduce per q-group
                # bf16 out is fine: tja only feeds b_ij whose error
                # is already bf16-matmul dominated
                with nc.allow_low_precision(reason="tja bf16"):
                    nc.vector.reduce_sum(
                        tjas[s3][:, qh * 8:qh * 8 + 8],
                        tmp8[:], axis=AX,
                    )

            # ---- chain piece: bd matmul + b update + softmax + wc for
            # k-chunk rows [k0, k1) of slice s3 (whole slice or one q) ----
            def chain_rows(s3, k0, k1, wct):
                n = k1 - k0
                bd = bpsum.tile([128, KSL * J], f32, name="bd", tag="bd")
                nc.tensor.matmul(
                    bd[:, :n * J], rep_sb[:],
                    tjas[s3][:, k0 - s3 * KSL:k1 - s3 * KSL].rearrange(
                        "p k j -> p (k j)"),
                    start=True, stop=True,
                )
                brs = b_rep[:, k0:k1]
                # b_rep += bd * 10/B  (x10 undoes the host-side W/10)
                nc.vector.scalar_tensor_tensor(
                    brs.rearrange("p k j -> p (k j)"),
                    bd[:, :n * J], 10.0 / B,
                    brs.rearrange("p k j -> p (k j)"),
                    op0=mult, op1=add,
                )
                ex = work.tile([128, KSL, J], f32, name="ex", tag="ex")
                nc.scalar.activation(ex[:, :n], brs, Exp)
                den = work.tile([128, KSL], f32, name="den", tag="den")
                nc.vector.reduce_sum(den[:, :n], ex[:, :n], axis=AX)
                rden = work.tile([128, KSL], f32, name="rden", tag="rden")
                nc.vector.reciprocal(rden[:, :n], den[:, :n])
                cb = work.tile([128, KSL, J], bf16, name="cb", tag="cb")
                # cb = (ex * 10) * (1/den)
                nc.vector.scalar_tensor_tensor(
                    cb[:, :n], ex[:, :n], 10.0,
                    rden[:, :n].unsqueeze(2).broadcast_to([128, n, J]),
                    op0=mult, op1=mult,
                )
                for q in range(k0 // 8, k1 // 8):
                    qh = q - s3 * QSL
                    r0 = qh * 8 - (k0 - s3 * KSL)
                    for h in range(2):
                        ce = cex.tile([128, 4, J, S], bf16,
                                      name="ce", tag="ce")
                        nc.scalar.copy(
                            ce[:],
                            cb[:, r0 + h * 4:r0 + h * 4 + 4].unsqueeze(3)
                              .broadcast_to([128, 4, J, S]),
                        )
                        nc.vector.tensor_mul(
                            wct[:, qh * 8 + h * 4:qh * 8 + h * 4 + 4],
                            wtf_t[q][:, h * 4:h * 4 + 4], ce[:],
                        )

            def chain(r, s3):
                wct = wcp.tile([128, KSL, J, S], bf16, name="wc", tag="wc")
                wc_slices[s3] = wct
                chain_rows(s3, s3 * KSL, (s3 + 1) * KSL, wct)

            def chain_q(r, s3, qh, wct):
                q = s3 * QSL + qh
                chain_rows(s3, q * 8, q * 8 + 8, wct)

            def tail():
                sb16 = work.tile([BSH, J, S], f32, name="sb16", tag="sb16")
                nc.vector.tensor_copy(
                    sb16[:],
                    sps[0][:BSH, :].rearrange("p (j t) -> p j t", t=S))
                sqb = work.tile([BSH, J, S], f32, name="sqb", tag="sqb")
                nc.vector.tensor_mul(sqb[:], sb16[:], sb16[:])
                msqb = work.tile([BSH, J], f32, name="msqb", tag="msqb")
                nc.vector.reduce_sum(msqb[:], sqb[:], axis=AX)
                d1b = work.tile([BSH, J], f32, name="d1b", tag="d1b")
                nc.vector.tensor_scalar_add(d1b[:], msqb[:], 1.0)
                rd1b = work.tile([BSH, J], f32, name="rd1b", tag="rd1b")
                nc.vector.reciprocal(rd1b[:], d1b[:])
                magb = work.tile([BSH, J], f32, name="magb", tag="magb")
                nc.scalar.sqrt(magb[:], msqb[:])
                f2b = work.tile([BSH, J], f32, name="f2b", tag="f2b")
                nc.vector.tensor_mul(f2b[:], magb[:], rd1b[:])
                nc.vector.tensor_mul(
                    vout[:].rearrange("p (j t) -> p j t", t=S),
                    sb16[:],
                    f2b[:].unsqueeze(2).broadcast_to([BSH, J, S]),
                )
                nc.sync.dma_start(out_d.ap(), vout[:])

            # ---- emission schedule (PE-stream software pipelining) ----
            for o in range(NQ):
                s_oct(0, o)
            cast_squash()                      # v_0

            m_q(0, 0, 0); m_q(0, 0, 1); m_q(0, 0, 2); chain(0, 0)
            m_q(0, 1, 0); m_q(0, 1, 1)
            m_q(0, 1, 2); chain(0, 1)
            issue_x2(1, 0, 3)
            s_oct(1, 0)
            m_q(0, 2, 0); s_oct(1, 1)
            m_q(0, 2, 1); s_oct(1, 2)
            m_q(0, 2, 2); chain(0, 2)
            issue_x2(1, 3, 6)
            nc.sync.dma_start(xtb_sb[:], xtb_d.ap())
            issue_x2(1, 6, NQ)
            s_oct(1, 3); s_oct(1, 4)
            s_oct(1, 5)
            s_oct(1, 6); s_oct(1, 7); s_oct(1, 8)
            cast_squash()                      # v_1

            m_q(1, 0, 0); m_q(1, 0, 1); m_q(1, 0, 2); chain(1, 0)
            m_q(1, 1, 0); m_q(1, 1, 1)
            m_q(1, 1, 2); chain(1, 1)
            s2_oct(0)
            m_q(1, 2, 0); s2_oct(1)
            m_q(1, 2, 1); s2_oct(2); s2_oct(3)
            m_q(1, 2, 2); chain(1, 2)
            s2_oct(4); s2_oct(5); s2_oct(6); s2_oct(7); s2_oct(8)
            tail()

    nc.compile()
    return nc


def _get_built():
    global _built
    if _built is None:
        _built = _build()
    return _built


def _prep_inputs(x, weight):
    x = np.asarray(x, dtype=np.float32)
    weight = np.asarray(weight, dtype=np.float32)
    xtf = np.ascontiguousarray(x.transpose(2, 1, 0)).reshape(KF, B)
    xtf_m = np.ascontiguousarray(
        xtf.reshape(NKF, 128, B).transpose(1, 0, 2)
    ).astype(_BF16)                       # [128, NKF, B]
    x2f = np.ascontiguousarray(x.transpose(0, 2, 1)).reshape(B, KF)
    x2f_m = np.ascontiguousarray(
        x2f.reshape(NB, 128, NQ, 8 * 128).transpose(1, 2, 0, 3)
    ).astype(_BF16)                       # [128, NQ, NB, 1024]
    wtf = np.ascontiguousarray(weight.transpose(0, 3, 1, 2)).reshape(KF, JT)
    wtf_m = np.ascontiguousarray(
        (wtf * 0.1).reshape(NKF, 128, J, S).transpose(1, 0, 2, 3)
    ).astype(_BF16)                       # [128, NKF, J, S]
    wtf_q = np.ascontiguousarray(
        wtf_m.reshape(128, NQ, 8, J, S))  # [128, NQ, 8, J, S]
    xtf_q = np.ascontiguousarray(
        xtf_m.reshape(128, NQ, 8, B))     # [128, NQ, 8, B]
    in_maps = []
    for core in range(NCORES):
        in_maps.append({
            "xtf": xtf_q,
            "wtf": wtf_q,
            "x2f": x2f_m,
            "xtb": np.ascontiguousarray(
                xtf_m[:, :, core * BSH:(core + 1) * BSH]),
        })
    return in_maps


def run(x, weight, trace=False, warmup=1, **kw):
    from concourse import bass_utils
    nc = _get_built()
    in_maps = _prep_inputs(x, weight)
    for _ in range(warmup):
        bass_utils.run_bass_kernel_spmd(
            nc, in_maps, core_ids=list(range(NCORES)), trace=False
        )
    res = bass_utils.run_bass_kernel_spmd(
        nc, in_maps, core_ids=list(range(NCORES)), trace=trace, **kw
    )
    out = np.concatenate(
        [np.asarray(res.results[c]["out"], dtype=np.float32)
         for c in range(NCORES)], axis=0,
    )
    return out.reshape(B, J, S, 1), res


def kernel(x, weight):
    out, _ = run(x, weight)
    return out
